# revision 1
# baseline (speedup 1.0000x reference)
"""Trainium2 Bass kernel for nn_CrossAttentionModel (cross-attention pooling).

Strategy
--------
Data-parallel over batch: core i handles batch item i (B=8, 8 cores, no
collectives).  The reference's huge [B,L2,L1,H]/[B,L2,L1,D] intermediates are
never materialized; instead the computation is refactored per pair (m,l):

    rh    = relu(H1[l] + H2[m] + tb1)          (H=1024)
    s     = rh @ W_a + b_a                     (W_a = tw2 @ aw1, folded)
    logit = relu(s) @ aw2 + ab2
    attn  = sigmoid(logit) * valid[m,l]
    P_h  += attn * rh ;  S += attn
    y     = (P_h . w_c + S * t_c) / (S + 1e-5) + cb   (w_c = tw2 @ cw)

which is algebraically identical to the reference (emb @ aw1 = rh @ (tw2@aw1)
+ tb2@aw1, and pooled.cw factors through tw2@cw).

On-chip layout keeps features on partitions and pairs on the free dim:
rh^T tiles [128, NP] feed the big s-matmul (fp32r, 1 cyc/row), the logit
matmul uses a column-replicated aw2 so sigmoid/masking/weighting run on all
128 partitions, and P_h/S are accumulated in exact fp32 on the vector engine.
H1/H2 are computed in true fp32 (value path is precision-sensitive).

Valid-pair compaction: rows l with mask1=0 and columns m with mask2=0
contribute nothing (attn=0), so the host compacts both index lists before
building the (static-shape) program; with Bernoulli(0.5) masks this cuts the
pair grid ~4x (capacities are maxed over the 8 cores, so ~2.3x in practice).
"""

import numpy as np

B, L1, L2, D, HH, V = 8, 64, 64, 768, 1024, 50257
PAD_ID = 50257
P = 128
DC = D // P    # 6 chunks of the 768 dims
HC = HH // P   # 8 chunks of the 1024 dims

_prog_cache = {}


def _build_program(N1, N2P, K, NBLK, ab2_f, cb_f, t_c_f, debug_stage=None):
    import concourse.bass as bass
    import concourse.bacc as bacc
    import concourse.mybir as mybir
    import concourse.tile as tile
    from concourse.masks import make_identity

    f32 = mybir.dt.float32
    bf16 = mybir.dt.bfloat16
    i32 = mybir.dt.int32
    Act = mybir.ActivationFunctionType
    Alu = mybir.AluOpType
    Axis = mybir.AxisListType

    NP = K * N1  # pairs per block
    NT = NBLK * NP

    nc = bacc.Bacc(
        "TRN2",
        target_bir_lowering=False,
        debug=False,
        enable_asserts=False,
        num_devices=8,
    )

    table = nc.dram_tensor("table", [V, D], f32, kind="ExternalInput").ap()
    idx1_d = nc.dram_tensor("idx1", [N1], i32, kind="ExternalInput").ap()
    idx2_d = nc.dram_tensor("idx2", [N2P], i32, kind="ExternalInput").ap()
    valid_d = nc.dram_tensor("valid", [NT], f32, kind="ExternalInput").ap()
    w1a_d = nc.dram_tensor("w1a", [D, HH], f32, kind="ExternalInput").ap()
    w1b_d = nc.dram_tensor("w1b", [D, HH], f32, kind="ExternalInput").ap()
    wa_d = nc.dram_tensor("W_a", [HH, D], bf16, kind="ExternalInput").ap()
    tb1_d = nc.dram_tensor("tb1v", [HH], f32, kind="ExternalInput").ap()
    ba_d = nc.dram_tensor("b_av", [D], f32, kind="ExternalInput").ap()
    aw2_d = nc.dram_tensor("aw2rep", [P, DC * P], bf16, kind="ExternalInput").ap()
    wc_d = nc.dram_tensor("w_cv", [HH], f32, kind="ExternalInput").ap()
    y_d = nc.dram_tensor("y", [1, 1], f32, kind="ExternalOutput").ap()
    dbg_d = None
    if debug_stage is not None:
        dbg_d = nc.dram_tensor("dbg", [P, 1024], f32, kind="ExternalOutput").ap()

    with tile.TileContext(nc, trace_sim=False) as tc:
        with (
            tc.tile_pool(name="const", bufs=1) as cpool,
            tc.tile_pool(name="wts", bufs=1) as wpool,
            tc.tile_pool(name="work", bufs=1) as work,
            tc.tile_pool(name="rh", bufs=1) as rhp,
            tc.tile_pool(name="ps", bufs=7, space="PSUM") as psp,
        ):
            ident = cpool.tile([P, P], f32)
            make_identity(nc, ident[:])
            ones_col = cpool.tile([P, 1], f32)
            nc.vector.memset(ones_col[:], 1.0)

            tb1c = cpool.tile([P, HC], f32)
            nc.sync.dma_start(tb1c[:], tb1_d.rearrange("(c p) -> p c", p=P))
            bac = cpool.tile([P, DC], f32)
            nc.sync.dma_start(bac[:], ba_d.rearrange("(c p) -> p c", p=P))
            wcc = cpool.tile([P, HC], f32)
            nc.sync.dma_start(wcc[:], wc_d.rearrange("(c p) -> p c", p=P))
            aw2rep = cpool.tile([P, DC * P], bf16)
            nc.sync.dma_start(aw2rep[:], aw2_d[:])
            valid_rep = cpool.tile([P, NT], f32)
            nc.sync.dma_start(
                valid_rep[:],
                valid_d.rearrange("(o t) -> o t", o=1).broadcast_to([P, NT]),
            )
            idx1_sb = cpool.tile([N1, 1], i32)
            nc.sync.dma_start(idx1_sb[:], idx1_d.rearrange("(p o) -> p o", o=1))
            idx2_sb = cpool.tile([N2P, 1], i32)
            nc.sync.dma_start(idx2_sb[:], idx2_d.rearrange("(p o) -> p o", o=1))

            w1a_sb = []
            w1b_sb = []
            wa_sb = []
            for dc in range(DC):
                t = wpool.tile([P, HH], f32, tag="w1a", bufs=DC, name=f"w1a_{dc}")
                nc.sync.dma_start(t[:], w1a_d[dc * P:(dc + 1) * P, :])
                w1a_sb.append(t)
            for dc in range(DC):
                t = wpool.tile([P, HH], f32, tag="w1b", bufs=DC, name=f"w1b_{dc}")
                nc.sync.dma_start(t[:], w1b_d[dc * P:(dc + 1) * P, :])
                w1b_sb.append(t)
            for hc in range(HC):
                t = wpool.tile([P, D], bf16, tag="wa", bufs=HC, name=f"wa_{hc}")
                nc.sync.dma_start(t[:], wa_d[hc * P:(hc + 1) * P, :])
                wa_sb.append(t)

            # ---- gather + transpose + first MLP (H1 = E1 @ w1a etc.) ----
            def build_HT(idx_sb, n, w_sb, label):
                E = work.tile([n, D], f32, tag=f"E_{label}", bufs=1, name=f"E_{label}")
                nc.gpsimd.indirect_dma_start(
                    out=E[:],
                    out_offset=None,
                    in_=table,
                    in_offset=bass.IndirectOffsetOnAxis(ap=idx_sb[:, :1], axis=0),
                )
                ET = []
                for dc in range(DC):
                    pt = psp.tile([P, n], f32, tag="ps", name=f"ptE_{label}_{dc}")
                    nc.tensor.transpose(pt[:], E[:, dc * P:(dc + 1) * P], ident[:n, :n])
                    t = work.tile([P, n], f32, tag=f"ET_{label}", bufs=DC,
                                  name=f"ET_{label}_{dc}")
                    nc.scalar.copy(t[:], pt[:])
                    ET.append(t)
                Hsb = work.tile([n, HH], f32, tag=f"H_{label}", bufs=1, name=f"H_{label}")
                for half in range(2):
                    ph = psp.tile([n, 512], f32, tag="ps", name=f"ph_{label}_{half}")
                    for dc in range(DC):
                        nc.tensor.matmul(
                            ph[:],
                            lhsT=ET[dc][:],
                            rhs=w_sb[dc][:, half * 512:(half + 1) * 512],
                            start=(dc == 0),
                            stop=(dc == DC - 1),
                        )
                    nc.scalar.copy(Hsb[:, half * 512:(half + 1) * 512], ph[:])
                HT = []
                for hc in range(HC):
                    pt2 = psp.tile([P, n], f32, tag="ps", name=f"ptH_{label}_{hc}")
                    nc.tensor.transpose(pt2[:], Hsb[:, hc * P:(hc + 1) * P], ident[:n, :n])
                    t = work.tile([P, n], f32, tag=f"HT_{label}", bufs=HC,
                                  name=f"HT_{label}_{hc}")
                    nc.scalar.copy(t[:], pt2[:])
                    HT.append(t)
                return HT

            stage = 99 if debug_stage is None else debug_stage
            if stage < 2:
                nc.sync.dma_start(dbg_d[:, 0:HC], tb1c[:])
                nc.sync.dma_start(dbg_d[:, 8:8 + 512], wa_sb[0][:, :512])
            if stage >= 2:
                H1T = build_HT(idx1_sb, N1, w1a_sb, "a")
                H2T = build_HT(idx2_sb, N2P, w1b_sb, "b")
            if stage == 2:
                nc.sync.dma_start(dbg_d[:, 0:N1], H1T[0][:])
                nc.sync.dma_start(dbg_d[:, 512:512 + N2P], H2T[0][:])

            if stage >= 6:
                S_parts = work.tile([P, NBLK], f32, tag="Sp", bufs=1)
            Ph_parts = []
            if stage >= 7:
                for hc in range(HC):
                    t = work.tile([P, NBLK], f32, tag="Php", bufs=HC, name=f"Php_{hc}")
                    Ph_parts.append(t)

            # ---- main pair-block loop ----
            for bi in range(NBLK if stage >= 3 else 0):
                rtfs = []
                rtbs = []
                for hc in range(HC):
                    rs = rhp.tile([P, NP], f32, tag="rs", bufs=HC + 1, name=f"rs_{bi}_{hc}")
                    nc.vector.tensor_tensor(
                        out=rs[:].rearrange("p (k l) -> p k l", k=K),
                        in0=H1T[hc][:].unsqueeze(1).broadcast_to([P, K, N1]),
                        in1=H2T[hc][:, bi * K:(bi + 1) * K].unsqueeze(2)
                            .broadcast_to([P, K, N1]),
                        op=Alu.add,
                    )
                    # fp32 relu copy (exact, for the P_h accumulation) on DVE
                    rtf = rhp.tile([P, NP], f32, tag="rtf", bufs=12, name=f"rtf_{bi}_{hc}")
                    nc.vector.tensor_scalar(
                        out=rtf[:], in0=rs[:],
                        scalar1=tb1c[:, hc:hc + 1], scalar2=0.0,
                        op0=Alu.add, op1=Alu.max,
                    )
                    # bf16 relu copy (matmul operand) on ACT
                    rtb = rhp.tile([P, NP], bf16, tag="rtb", bufs=12, name=f"rtb_{bi}_{hc}")
                    nc.scalar.activation(rtb[:], rs[:], Act.Relu, bias=tb1c[:, hc:hc + 1])
                    rtfs.append(rtf)
                    rtbs.append(rtb)

                if stage == 3:
                    if bi == 0:
                        nc.sync.dma_start(dbg_d[:, 0:NP], rtfs[0][:])
                    continue
                ats = []
                for dc in range(DC):
                    ps = psp.tile([P, NP], f32, tag="ps", name=f"ps_s_{bi}_{dc}")
                    for hc in range(HC):
                        nc.tensor.matmul(
                            ps[:],
                            lhsT=wa_sb[hc][:, dc * P:(dc + 1) * P],
                            rhs=rtbs[hc][:],
                            start=(hc == 0),
                            stop=(hc == HC - 1),
                        )
                    at = rhp.tile([P, NP], bf16, tag="at", bufs=DC + 1, name=f"at_{bi}_{dc}")
                    nc.scalar.activation(at[:], ps[:], Act.Relu, bias=bac[:, dc:dc + 1])
                    ats.append(at)

                if stage == 4:  # after s-matmul+at
                    if bi == 0:
                        atf = rhp.tile([P, NP], f32, tag="atf", bufs=1)
                        nc.vector.tensor_copy(atf[:], ats[0][:])
                        nc.sync.dma_start(dbg_d[:, 0:NP], atf[:])
                    continue
                pl = psp.tile([P, NP], f32, tag="ps", name=f"pl_{bi}")
                for dc in range(DC):
                    nc.tensor.matmul(
                        pl[:],
                        lhsT=aw2rep[:, dc * P:(dc + 1) * P],
                        rhs=ats[dc][:],
                        start=(dc == 0),
                        stop=(dc == DC - 1),
                    )
                attn = rhp.tile([P, NP], f32, tag="attn", bufs=2, name=f"attn_{bi}")
                nc.scalar.activation(attn[:], pl[:], Act.Sigmoid, bias=float(ab2_f))
                if stage == 5:  # after logit+sigmoid
                    if bi == 0:
                        nc.sync.dma_start(dbg_d[:, 0:NP], attn[:])
                    continue
                attnm = rhp.tile([P, NP], f32, tag="attnm", bufs=2, name=f"attnm_{bi}")
                nc.vector.tensor_tensor(
                    out=attnm[:], in0=attn[:],
                    in1=valid_rep[:, bi * NP:(bi + 1) * NP], op=Alu.mult,
                )
                nc.vector.tensor_reduce(
                    out=S_parts[:, bi:bi + 1], in_=attnm[:], axis=Axis.X, op=Alu.add,
                )
                if stage == 6:
                    if bi == 0:
                        nc.sync.dma_start(dbg_d[:, 0:NP], attnm[:])
                    continue
                for hc in range(HC):
                    scr = rhp.tile([P, NP], f32, tag="scr", bufs=3, name=f"scr_{bi}_{hc}")
                    nc.vector.tensor_tensor(
                        out=scr[:], in0=rtfs[hc][:], in1=attnm[:], op=Alu.mult,
                    )
                    nc.vector.tensor_reduce(
                        out=Ph_parts[hc][:, bi:bi + 1], in_=scr[:],
                        axis=Axis.X, op=Alu.add,
                    )

            # ---- final reduction: y = (P_h.w_c + S*t_c)/(S+1e-5) + cb ----
            if stage >= 7:
                Ph_all = work.tile([P, HC], f32, tag="Phall", bufs=1)
                for hc in range(HC):
                    nc.vector.tensor_reduce(
                        out=Ph_all[:, hc:hc + 1], in_=Ph_parts[hc][:],
                        axis=Axis.X, op=Alu.add,
                    )
                S_vec = work.tile([P, 1], f32, tag="Svec", bufs=1)
                nc.vector.tensor_reduce(out=S_vec[:], in_=S_parts[:], axis=Axis.X,
                                        op=Alu.add)
                yn_scr = work.tile([P, HC], f32, tag="ynscr", bufs=1)
                yn_vec = work.tile([P, 1], f32, tag="ynvec", bufs=1)
                nc.vector.tensor_tensor(out=yn_scr[:], in0=Ph_all[:], in1=wcc[:],
                                        op=Alu.mult)
                nc.vector.tensor_reduce(out=yn_vec[:], in_=yn_scr[:],
                                        axis=Axis.X, op=Alu.add)
                psy = psp.tile([1, 1], f32, tag="ps", name="psy")
                nc.tensor.matmul(psy[:], lhsT=yn_vec[:], rhs=ones_col[:],
                                 start=True, stop=True)

                den = work.tile([1, 1], f32, tag="den", bufs=1)
                nc.vector.tensor_scalar_add(den[:], S_vec[0:1, :], 1e-5)
                rden = work.tile([1, 1], f32, tag="rden", bufs=1)
                nc.vector.reciprocal(rden[:], den[:])
                num = work.tile([1, 1], f32, tag="num", bufs=1)
                nc.vector.scalar_tensor_tensor(
                    out=num[:], in0=S_vec[0:1, :], scalar=float(t_c_f), in1=psy[:],
                    op0=Alu.mult, op1=Alu.add,
                )
                y0 = work.tile([1, 1], f32, tag="y0", bufs=1)
                nc.vector.tensor_tensor(out=y0[:], in0=num[:], in1=rden[:], op=Alu.mult)
                y1 = work.tile([1, 1], f32, tag="y1", bufs=1)
                nc.vector.tensor_scalar_add(y1[:], y0[:], float(cb_f))
                nc.sync.dma_start(y_d[:], y1[:])

    nc.compile()
    return nc


def _prep(x1, x2, mask1, mask2, embed_table, tw1, tb1, tw2, tb2,
          aw1, ab1, aw2, ab2, cw, cb, compact=True):
    """Host-side sharding/index prep. Returns (program args, per-core in_maps)."""
    import ml_dtypes
    f32 = np.float32
    bf16 = ml_dtypes.bfloat16
    x1 = np.where(x1 == PAD_ID, 0, x1).astype(np.int32)
    x2 = np.where(x2 == PAD_ID, 0, x2).astype(np.int32)
    w1a = np.ascontiguousarray(tw1[:D]).astype(f32)
    w1b = np.ascontiguousarray(tw1[D:]).astype(f32)
    W_a = (tw2.astype(np.float64) @ aw1.astype(np.float64)).astype(f32)
    b_a = (tb2.astype(np.float64) @ aw1.astype(np.float64)
           + ab1.astype(np.float64)).astype(f32)
    w_c = (tw2.astype(np.float64) @ cw.astype(np.float64)).astype(f32).ravel()
    t_c = float(tb2.astype(np.float64) @ cw.astype(np.float64).ravel())

    if compact:
        l_lists = [np.nonzero(mask1[b])[0] for b in range(B)]
        m_lists = [np.nonzero(mask2[b])[0] for b in range(B)]
        N1 = max(4, max(len(l) for l in l_lists))
        N1 = (N1 + 3) & ~3
        N2 = max(1, max(len(m) for m in m_lists))
    else:
        l_lists = [np.arange(L1) for _ in range(B)]
        m_lists = [np.arange(L2) for _ in range(B)]
        N1, N2 = L1, L2
    K = max(1, min(16, 512 // N1))
    NBLK = -(-N2 // K)
    N2P = NBLK * K
    NP = K * N1
    NT = NBLK * NP

    table_f32 = np.ascontiguousarray(embed_table, dtype=f32)
    # aw2 replicated across matmul output columns: aw2rep[k, c*128+m] = aw2[c*128+k]
    aw2rep_host = np.ascontiguousarray(np.broadcast_to(
        aw2.astype(f32).ravel().reshape(DC, P).T[:, :, None], (P, DC, P)
    ).reshape(P, DC * P)).astype(bf16)
    in_maps = []
    for b in range(B):
        ll, ml = l_lists[b], m_lists[b]
        idx1 = np.zeros(N1, np.int32)
        idx1[:len(ll)] = x1[b][ll]
        idx2 = np.zeros(N2P, np.int32)
        idx2[:len(ml)] = x2[b][ml]
        valid = np.zeros((N2P, N1), f32)
        if len(ll) and len(ml):
            vm = (mask1[b][ll][None, :] != 0) & (mask2[b][ml][:, None] != 0) \
                 & (x1[b][ll][None, :] != x2[b][ml][:, None])
            valid[:len(ml), :len(ll)] = vm.astype(f32)
        in_maps.append({
            "table": table_f32,
            "idx1": idx1,
            "idx2": idx2,
            "valid": valid.ravel(),
            "w1a": w1a,
            "w1b": w1b,
            "W_a": W_a.astype(bf16),
            "tb1v": tb1.astype(f32),
            "b_av": b_a,
            "aw2rep": aw2rep_host,
            "w_cv": w_c,
        })
    ab2_f = float(np.asarray(ab2).ravel()[0])
    cb_f = float(np.asarray(cb).ravel()[0])
    return (N1, N2P, K, NBLK, ab2_f, cb_f, t_c), in_maps


def kernel(x1, x2, mask1, mask2, embed_table, tw1, tb1, tw2, tb2,
           aw1, ab1, aw2, ab2, cw, cb):
    from concourse import bass_utils

    (N1, N2P, K, NBLK, ab2_f, cb_f, t_c), in_maps = _prep(
        x1, x2, mask1, mask2, embed_table, tw1, tb1, tw2, tb2,
        aw1, ab1, aw2, ab2, cw, cb)

    key = (N1, N2P, K, NBLK, ab2_f, cb_f, t_c)
    if key not in _prog_cache:
        _prog_cache[key] = _build_program(*key)
    nc = _prog_cache[key]

    res = bass_utils.run_bass_kernel_spmd(nc, in_maps, core_ids=list(range(8)))
    y = np.stack([res.results[i]["y"].reshape(()) for i in range(B)])
    return y.reshape(B, 1).astype(np.float32)



# revision 23
# speedup vs baseline: 2.2210x; 2.2210x over previous
"""Trainium2 Bass kernel for nn_CrossAttentionModel (cross-attention pooling).

Strategy (v2)
-------------
Data-parallel over batch: core i handles batch item i (B=8, 8 cores, no
collectives).  Host folds the weight chain and precomputes the tiny
per-sequence H matrices; the device runs the O(NT*H) pair grid, which is
>95% of the FLOPs:

    H1 = e1 @ w1a              (host, [n1,H])
    H2 = e2 @ w1b + tb1        (host, [n2,H])
    rs    = H1[l] + H2[m]                      DVE  (fp16)
    rhv   = relu(rs) * 64                      DVE  (fp16, value path)
    rtb   = fp8(relu(rs) * 64)                 ACT  (one fused pass)
    s     = rtb @ (32*W_a.fp8)                 PE   fp8 DoubleRow (2 col/cyc)
    at    = fp8(128*relu(s + b_a))             ACT
    logit = at @ (32*aw2.fp8) + maskbias       PE   fp8 DoubleRow + bf16 rank-1
    attnm = sigmoid(logit/4096 + ab2)          ACT  (accum_out -> S)
    v     = rhv @ (512*w_c as 2 fp16 limbs)    PE   (exactness via limb split)
    Pw   += sum(attnm * v)                     DVE  tensor_tensor_reduce
    y     = (Pw/(64*512) + S*t_c)/(S+1e-5)+cb

W_a = tw2 @ aw1 folds the trans-MLP second layer into the attn MLP
(emb @ aw1 = rh @ W_a + tb2@aw1), and w_c = tw2 @ cw projects the pooled
value so only the scalar per-pair projection v_p = rh_p . w_c is needed
(pooled @ cw = (sum attn*rh) @ w_c / denom + ...).  The fp8 attention path
is safe because logits are tiny (sigmoid ~ 0.5 + logit/4); the value path
needs w_c kept near-exact (shared quantization noise does not average out
over pairs), hence the two-limb fp16 split of w_c on the PE.

Valid-pair compaction as in v1: masked-out rows/cols dropped on host;
invalid + padded pairs get a -1e6 logit bias (added in PSUM via a bf16
rank-1 matmul), so sigmoid drives them to exactly 0.
"""

import numpy as np

B, L1, L2, D, HH, V = 8, 64, 64, 768, 1024, 50257
PAD_ID = 50257
P = 128
DC = D // P    # 6 chunks of the 768 attn dims
HC = HH // P   # 8 chunks of the 1024 hidden dims
HP = HC // 2   # 4 DoubleRow h-pair groups
DP = DC // 2   # 3 DoubleRow d-pair groups

SC_RH = 64.0    # rhv / rtb scale
SC_WA = 32.0    # W_a fp8 scale
SC_AT = 128.0   # at fp8 scale
SC_A2 = 32.0    # aw2 fp8 scale
SC_WC = 512.0   # w_c limb scale
MASKV = -1.0e6 * SC_AT * SC_A2   # pre-sigmoid-descale mask bias in PSUM units

_prog_cache = {}


def _build_program(N1, N2P, K, NBLK, ab2_f, cb_f, t_c_f,
                   use_dr=True, use_ttr=0, stage=8):
    # use_ttr: tensor_tensor_reduce on single-partition rows crashes the
    # exec unit on TRN2 hardware (sim-only feature here); keep 0 (TT + TR).
    import concourse.bass as bass
    import concourse.bacc as bacc
    import concourse.mybir as mybir
    import concourse.tile as tile

    f32 = mybir.dt.float32
    f16 = mybir.dt.float16
    bf16 = mybir.dt.bfloat16
    f8 = mybir.dt.float8e4
    Act = mybir.ActivationFunctionType
    Alu = mybir.AluOpType
    Axis = mybir.AxisListType
    DR = mybir.MatmulPerfMode.DoubleRow

    NP = K * N1                 # pairs per block
    NPS = (NP + 15) & ~15       # fp8 plane stride (mult of 16)

    nc = bacc.Bacc(
        "TRN2",
        target_bir_lowering=False,
        debug=False,
        enable_asserts=False,
        num_devices=8,
    )

    h1t_d = nc.dram_tensor("h1t", [P, HC * N1], f16, kind="ExternalInput").ap()
    h2t_d = nc.dram_tensor("h2t", [P, HC * N2P], f16, kind="ExternalInput").ap()
    wa_d = nc.dram_tensor("wa", [HP * P, 2 * D], f8, kind="ExternalInput").ap()
    bac_d = nc.dram_tensor("bac", [P, DC], f32, kind="ExternalInput").ap()
    aw2_d = nc.dram_tensor("aw2", [P, DP * 2 * 16], f8, kind="ExternalInput").ap()
    wcl_d = nc.dram_tensor("wcl", [P, 2 * HC], f16, kind="ExternalInput").ap()
    vld_d = nc.dram_tensor("vld", [1, NBLK * NPS], f32, kind="ExternalInput").ap()
    y_d = nc.dram_tensor("y", [1, 1], f32, kind="ExternalOutput").ap()

    with tile.TileContext(nc, trace_sim=False) as tc:
        with (
            tc.tile_pool(name="const", bufs=1) as cpool,
            tc.tile_pool(name="work", bufs=1) as work,
            tc.tile_pool(name="ps", bufs=4, space="PSUM") as psp,
            tc.tile_pool(name="psv", bufs=2, space="PSUM") as psv,
        ):
            h1t = cpool.tile([P, HC, N1], f16)
            nc.sync.dma_start(h1t[:].rearrange("p h l -> p (h l)"), h1t_d[:])
            h2t = cpool.tile([P, HC, N2P], f16)
            nc.sync.dma_start(h2t[:].rearrange("p h m -> p (h m)"), h2t_d[:])
            wa_sb = []
            for hp in range(HP):
                t = cpool.tile([P, 2, D], f8, tag="wa", bufs=HP, name=f"wa_{hp}")
                nc.sync.dma_start(
                    t[:].rearrange("p j d -> p (j d)"),
                    wa_d[hp * P:(hp + 1) * P, :],
                )
                wa_sb.append(t)
            bac = cpool.tile([P, DC], f32)
            nc.sync.dma_start(bac[:], bac_d[:])
            aw2 = cpool.tile([P, DP, 2, 16], f8)
            nc.sync.dma_start(aw2[:].rearrange("p a j s -> p (a j s)"), aw2_d[:])
            wcl = cpool.tile([P, 2 * HC], f16)
            nc.sync.dma_start(wcl[:], wcl_d[:])
            vld = cpool.tile([1, NBLK * NPS], f32)
            nc.sync.dma_start(vld[:], vld_d[:])

            Pw_parts = work.tile([1, NBLK], f32, tag="pwp", bufs=1)
            S_parts = work.tile([1, NBLK], f32, tag="sp", bufs=1)
            s_accs = [None] * (NBLK + 1)
            pw_accs = [None] * (NBLK + 1)
            if use_ttr == 2:
                zz = work.tile([1, 1], f32, tag="zz", bufs=1)
                nc.vector.memset(zz[:], 0.0)
                s_accs[0] = zz
                pw_accs[0] = zz

            for bi in range(NBLK):
                # rs = H1[l] + H2[m]  (fp16, [P, HC, K*N1])
                rs = work.tile([P, HC, NP], f16, tag="rs", bufs=2, name=f"rs{bi}")
                for hc in range(HC):
                    nc.vector.tensor_tensor(
                        out=rs[:, hc, :].rearrange("p (k l) -> p k l", k=K),
                        in0=h1t[:, hc, :].unsqueeze(1).broadcast_to([P, K, N1]),
                        in1=h2t[:, hc, bi * K:(bi + 1) * K].unsqueeze(2)
                            .broadcast_to([P, K, N1]),
                        op=Alu.add,
                    )
                if stage < 2:
                    continue
                # rtb = fp8(64*relu(rs)) in DoubleRow layout [P, HC, NPS]
                rtb = work.tile([P, HC, NPS], f8, tag="rtb", bufs=2, name=f"rtb{bi}")
                nc.scalar.activation(
                    rtb[:, :, :NP], rs[:], Act.Relu, scale=SC_RH,
                )
                if stage < 3:
                    continue
                # rhv = 64*relu(rs) fp16 (value path)
                rhv = work.tile([P, HC, NP], f16, tag="rhv", bufs=2, name=f"rhv{bi}")
                for hc in range(HC):
                    nc.vector.tensor_scalar(
                        out=rhv[:, hc, :], in0=rs[:, hc, :],
                        scalar1=0.0, scalar2=SC_RH,
                        op0=Alu.max, op1=Alu.mult,
                    )

                if stage < 4:
                    continue
                # s = rtb @ W_a  (fp8 DoubleRow), at = fp8(128*relu(s+b_a))
                at = work.tile([P, DC, NPS], f8, tag="at", bufs=2, name=f"at{bi}")
                for dc in range(DC):
                    ps = psp.tile([P, NP], f32, tag="ps", name=f"ps{bi}_{dc}")
                    if use_dr:
                        for hp in range(HP):
                            nc.tensor.matmul(
                                ps[:],
                                lhsT=wa_sb[hp][:, :, dc * P:(dc + 1) * P],
                                rhs=rtb[:, 2 * hp:2 * hp + 2, :NP],
                                start=(hp == 0),
                                stop=(hp == HP - 1),
                                perf_mode=DR,
                            )
                    else:
                        for hc in range(HC):
                            nc.tensor.matmul(
                                ps[:],
                                lhsT=wa_sb[hc // 2][:, hc % 2, dc * P:(dc + 1) * P],
                                rhs=rtb[:, hc, :NP],
                                start=(hc == 0),
                                stop=(hc == HC - 1),
                            )
                    nc.scalar.activation(
                        at[:, dc, :NP], ps[:], Act.Relu,
                        bias=bac[:, dc:dc + 1], scale=SC_AT / (SC_RH * SC_WA),
                    )

                if stage < 5:
                    continue
                # logit = at @ aw2  ([1, NP] PSUM)
                pl = psv.tile([1, NP], f32, tag="pl", bufs=2, name=f"pl{bi}")
                if use_dr:
                    for dp in range(DP):
                        nc.tensor.matmul(
                            pl[:],
                            lhsT=aw2[:, dp, :, 0:1],
                            rhs=at[:, 2 * dp:2 * dp + 2, :NP],
                            start=(dp == 0),
                            stop=(dp == DP - 1),
                            perf_mode=DR,
                        )
                else:
                    for dc in range(DC):
                        nc.tensor.matmul(
                            pl[:],
                            lhsT=aw2[:, dc // 2, dc % 2, 0:1],
                            rhs=at[:, dc, :NP],
                            start=(dc == 0),
                            stop=(dc == DC - 1),
                        )
                # attn = sigmoid(logit/4096 + ab2)
                attn = work.tile([1, NP], f32, tag="attn", bufs=2, name=f"attn{bi}")
                nc.scalar.activation(
                    attn[:], pl[:], Act.Sigmoid,
                    bias=float(ab2_f), scale=1.0 / (SC_AT * SC_A2),
                )
                if stage < 6:
                    continue
                # attnm = attn * valid; accum -> S
                attnm = work.tile([1, NP], f32, tag="attnm", bufs=2, name=f"attnm{bi}")
                if use_ttr == 2:
                    s_accs[bi + 1] = work.tile([1, 1], f32, tag="sacc", bufs=NBLK,
                                               name=f"sacc{bi}")
                    nc.vector.tensor_tensor_reduce(
                        out=attnm[:], in0=attn[:],
                        in1=vld[:, bi * NPS:bi * NPS + NP],
                        scale=1.0, scalar=s_accs[bi][:],
                        op0=Alu.mult, op1=Alu.add,
                        accum_out=s_accs[bi + 1][:],
                    )
                elif use_ttr:
                    nc.vector.tensor_tensor_reduce(
                        out=attnm[:], in0=attn[:],
                        in1=vld[:, bi * NPS:bi * NPS + NP],
                        scale=1.0, scalar=0.0,
                        op0=Alu.mult, op1=Alu.add,
                        accum_out=S_parts[:, bi:bi + 1],
                    )
                else:
                    nc.vector.tensor_tensor(
                        out=attnm[:], in0=attn[:],
                        in1=vld[:, bi * NPS:bi * NPS + NP], op=Alu.mult,
                    )
                    nc.vector.tensor_reduce(
                        out=S_parts[:, bi:bi + 1], in_=attnm[:],
                        axis=Axis.X, op=Alu.add,
                    )

                if stage < 7:
                    continue
                # v = rhv @ (wcl0 + wcl1)  ([1, NP] PSUM, limbs accumulate)
                pv = psv.tile([1, NP], f32, tag="pv", bufs=2, name=f"pv{bi}")
                for j in range(2):
                    for hc in range(HC):
                        nc.tensor.matmul(
                            pv[:],
                            lhsT=wcl[:, j * HC + hc:j * HC + hc + 1],
                            rhs=rhv[:, hc, :],
                            start=(j == 0 and hc == 0),
                            stop=(j == 1 and hc == HC - 1),
                        )
                if stage < 8:
                    continue
                # Pw_parts[bi] = sum(attnm * v)
                vm = work.tile([1, NP], f32, tag="vm", bufs=2, name=f"vm{bi}")
                if use_ttr == 2:
                    pw_accs[bi + 1] = work.tile([1, 1], f32, tag="pwacc", bufs=NBLK,
                                                name=f"pwacc{bi}")
                    nc.vector.tensor_tensor_reduce(
                        out=vm[:], in0=attnm[:], in1=pv[:],
                        scale=1.0, scalar=pw_accs[bi][:],
                        op0=Alu.mult, op1=Alu.add,
                        accum_out=pw_accs[bi + 1][:],
                    )
                elif use_ttr:
                    nc.vector.tensor_tensor_reduce(
                        out=vm[:], in0=attnm[:], in1=pv[:],
                        scale=1.0, scalar=0.0,
                        op0=Alu.mult, op1=Alu.add,
                        accum_out=Pw_parts[:, bi:bi + 1],
                    )
                else:
                    nc.vector.tensor_tensor(
                        out=vm[:], in0=attnm[:], in1=pv[:], op=Alu.mult,
                    )
                    nc.vector.tensor_reduce(
                        out=Pw_parts[:, bi:bi + 1], in_=vm[:],
                        axis=Axis.X, op=Alu.add,
                    )

            # ---- final: y = (Pw/(64*512) + S*t_c)/(S+1e-5) + cb ----
            if stage < 8:
                ydum = work.tile([1, 1], f32, tag="ydum", bufs=1)
                nc.vector.memset(ydum[:], 0.0)
                nc.sync.dma_start(y_d[:], ydum[:])
            if stage >= 8:
                if use_ttr == 2:
                    Pw = pw_accs[NBLK]
                    S = s_accs[NBLK]
                else:
                    Pw = work.tile([1, 1], f32, tag="pw", bufs=1)
                    nc.vector.tensor_reduce(out=Pw[:], in_=Pw_parts[:], axis=Axis.X,
                                            op=Alu.add)
                    S = work.tile([1, 1], f32, tag="s", bufs=1)
                    nc.vector.tensor_reduce(out=S[:], in_=S_parts[:], axis=Axis.X,
                                            op=Alu.add)
                den = work.tile([1, 1], f32, tag="den", bufs=1)
                nc.vector.tensor_scalar_add(den[:], S[:], 1e-5)
                rden = work.tile([1, 1], f32, tag="rden", bufs=1)
                nc.vector.reciprocal(rden[:], den[:])
                num = work.tile([1, 1], f32, tag="num", bufs=1)
                # num = Pw/(64*512) + S*t_c
                pw_sc = work.tile([1, 1], f32, tag="pwsc", bufs=1)
                nc.vector.tensor_scalar_mul(pw_sc[:], Pw[:], 1.0 / (SC_RH * SC_WC))
                nc.vector.scalar_tensor_tensor(
                    out=num[:], in0=S[:], scalar=float(t_c_f), in1=pw_sc[:],
                    op0=Alu.mult, op1=Alu.add,
                )
                y0 = work.tile([1, 1], f32, tag="y0", bufs=1)
                nc.vector.tensor_tensor(out=y0[:], in0=num[:], in1=rden[:],
                                        op=Alu.mult)
                y1 = work.tile([1, 1], f32, tag="y1", bufs=1)
                nc.vector.tensor_scalar_add(y1[:], y0[:], float(cb_f))
                nc.sync.dma_start(y_d[:], y1[:])

    nc.compile()
    return nc


def _prep(x1, x2, mask1, mask2, embed_table, tw1, tb1, tw2, tb2,
          aw1, ab1, aw2, ab2, cw, cb, compact=True):
    """Host-side prep: compaction, weight folding, H matmuls, per-core maps."""
    import ml_dtypes
    f32 = np.float32
    f16 = np.float16
    bf16 = ml_dtypes.bfloat16
    f8 = ml_dtypes.float8_e4m3fn
    f64 = np.float64

    x1 = np.where(x1 == PAD_ID, 0, x1).astype(np.int32)
    x2 = np.where(x2 == PAD_ID, 0, x2).astype(np.int32)
    w1a = np.ascontiguousarray(tw1[:D]).astype(f64)
    w1b = np.ascontiguousarray(tw1[D:]).astype(f64)
    W_a = (tw2.astype(f64) @ aw1.astype(f64)).astype(f32)
    b_a = (tb2.astype(f64) @ aw1.astype(f64) + ab1.astype(f64)).astype(f32)
    w_c = (tw2.astype(f64) @ cw.astype(f64)).astype(f32).ravel()
    t_c = float(tb2.astype(f64) @ cw.astype(f64).ravel())

    if compact:
        l_lists = [np.nonzero(mask1[b])[0] for b in range(B)]
        m_lists = [np.nonzero(mask2[b])[0] for b in range(B)]
        N1 = max(4, max((len(l) for l in l_lists), default=4))
        N1 = (N1 + 3) & ~3
        N2 = max(1, max((len(m) for m in m_lists), default=1))
    else:
        l_lists = [np.arange(L1) for _ in range(B)]
        m_lists = [np.arange(L2) for _ in range(B)]
        N1, N2 = L1, L2
    K_max = max(1, min(16, 512 // N1))
    NBLK = -(-N2 // K_max)
    K = -(-N2 // NBLK)          # shrink K to just cover N2 in NBLK blocks
    N2P = NBLK * K
    NP = K * N1
    NPS = (NP + 15) & ~15

    # shared (per-core identical) weight tensors
    wa_host = np.empty((HP * P, 2 * D), dtype=f8)
    for hp in range(HP):
        for j in range(2):
            # wa_host[hp*P+p, j*D+d] = SC_WA * W_a[(2hp+j)*128+p, d]
            wa_host[hp * P:(hp + 1) * P, j * D:(j + 1) * D] = \
                (SC_WA * W_a[(2 * hp + j) * P:(2 * hp + j + 1) * P, :]).astype(f8)
    bac_host = np.ascontiguousarray(
        (SC_AT * b_a).reshape(DC, P).T, dtype=f32)
    aw2_host = np.zeros((P, DP, 2, 16), dtype=f8)
    a2 = (SC_A2 * aw2.astype(f32).ravel()).reshape(DC, P)  # [dc, p]
    for dp in range(DP):
        for j in range(2):
            aw2_host[:, dp, j, 0] = a2[2 * dp + j, :].astype(f8)
    aw2_host = np.ascontiguousarray(aw2_host.reshape(P, DP * 2 * 16))
    wc_s = (SC_WC * w_c).astype(f32)
    l0 = wc_s.astype(f16)
    l1 = (wc_s - l0.astype(f32)).astype(f16)
    wcl_host = np.empty((P, 2 * HC), dtype=f16)
    wcl_host[:, :HC] = l0.reshape(HC, P).T
    wcl_host[:, HC:] = l1.reshape(HC, P).T

    table = np.asarray(embed_table, dtype=f32)
    in_maps = []
    for b in range(B):
        ll, ml = l_lists[b], m_lists[b]
        n1, n2 = len(ll), len(ml)
        # host H matmuls (fp64 for exactness, stored fp16 transposed)
        h1t = np.zeros((P, HC * N1), dtype=f16)
        h2t = np.zeros((P, HC * N2P), dtype=f16)
        if n1:
            e1 = table[x1[b][ll]].astype(f64)          # [n1, D]
            H1 = (e1 @ w1a).astype(f32)                # [n1, HH]
            h1 = H1.T.reshape(HC, P, n1)               # [hc, p, l]
            h1t.reshape(P, HC, N1)[:, :, :n1] = \
                np.transpose(h1, (1, 0, 2)).astype(f16)
        if n2:
            e2 = table[x2[b][ml]].astype(f64)
            H2 = (e2 @ w1b + tb1.astype(f64)).astype(f32)
            h2 = H2.T.reshape(HC, P, n2)
            h2t.reshape(P, HC, N2P)[:, :, :n2] = \
                np.transpose(h2, (1, 0, 2)).astype(f16)
        # valid rows: 1 where the pair contributes, 0 elsewhere (incl. padding)
        vld = np.zeros((NBLK, NPS), dtype=f32)
        if n1 and n2:
            vm = (mask1[b][ll][None, :] != 0) & (mask2[b][ml][:, None] != 0) \
                 & (x1[b][ll][None, :] != x2[b][ml][:, None])   # [n2, n1]
            grid = np.zeros((N2P, N1), dtype=f32)
            grid[:n2, :n1] = vm.astype(f32)
            vld[:, :NP] = grid.reshape(NBLK, K * N1)
        in_maps.append({
            "h1t": h1t,
            "h2t": h2t,
            "wa": wa_host,
            "bac": bac_host,
            "aw2": aw2_host,
            "wcl": wcl_host,
            "vld": vld.reshape(1, NBLK * NPS),
        })
    ab2_f = float(np.asarray(ab2).ravel()[0])
    cb_f = float(np.asarray(cb).ravel()[0])
    return (N1, N2P, K, NBLK, ab2_f, cb_f, t_c), in_maps


def kernel(x1, x2, mask1, mask2, embed_table, tw1, tb1, tw2, tb2,
           aw1, ab1, aw2, ab2, cw, cb):
    from concourse import bass_utils

    key_args, in_maps = _prep(
        x1, x2, mask1, mask2, embed_table, tw1, tb1, tw2, tb2,
        aw1, ab1, aw2, ab2, cw, cb)

    if key_args not in _prog_cache:
        _prog_cache[key_args] = _build_program(*key_args)
    nc = _prog_cache[key_args]

    res = bass_utils.run_bass_kernel_spmd(nc, in_maps, core_ids=list(range(8)))
    y = np.stack([res.results[i]["y"].reshape(()) for i in range(B)])
    return y.reshape(B, 1).astype(np.float32)


# revision 38
# speedup vs baseline: 2.2286x; 1.0035x over previous
"""Trainium2 Bass kernel for nn_CrossAttentionModel (cross-attention pooling).

Strategy (v3)
-------------
Data-parallel over batch: core i handles batch item i (B=8, 8 cores, no
collectives).  Host folds the weight chain and precomputes the tiny
per-sequence H matrices; the device runs the O(NT*H) pair grid, which is
>95% of the FLOPs:

    H1 = e1 @ w1a              (host, [n1,H])
    H2 = e2 @ w1b + tb1        (host, [n2,H])
    rs    = H1[l] + H2[m]                      DVE  (fp16)
    rhv   = relu(rs) * 64                      DVE  (fp16, value path)
    rtb   = fp8(relu(rs) * 64)                 ACT  (one fused pass)
    s     = rtb @ (32*W_a.fp8)                 PE   fp8 DoubleRow (2 col/cyc)
    at    = fp8(128*relu(s + b_a))             ACT
    logit = at @ (32*aw2.fp8)                  PE   fp8 DoubleRow
    attn  = sigmoid(logit/4096 + ab2)          ACT
    attnm = attn * valid                       DVE  (+ TR -> S)
    v     = rhv @ (512*w_c as 2 fp16 limbs)    PE   (one [128,2] stationary)
    Pw   += sum(attnm * (v0+v1))               DVE
    y     = (Pw/(64*512) + S*t_c)/(S+1e-5)+cb

W_a = tw2 @ aw1 folds the trans-MLP second layer into the attn MLP
(emb @ aw1 = rh @ W_a + tb2@aw1), and w_c = tw2 @ cw projects the pooled
value so only the scalar per-pair projection v_p = rh_p . w_c is needed
(pooled @ cw = (sum attn*rh) @ w_c / denom + ...).  The fp8 attention path
is safe because logits are tiny (sigmoid ~ 0.5 + logit/4); the value path
needs w_c kept near-exact (shared quantization noise does not average out
over pairs), hence the two-limb fp16 split of w_c on the PE.

HW notes: tensor_tensor_reduce on single-partition rows crashes the TRN2
exec unit (use TT+TR); DoubleRow lhsT plane strides must be 16B-aligned;
a dummy-matmul chain at program start warms the PE clock gate during the
runtime preamble + input DMAs.
"""

import numpy as np

B, L1, L2, D, HH, V = 8, 64, 64, 768, 1024, 50257
PAD_ID = 50257
P = 128
DC = D // P    # 6 chunks of the 768 attn dims
HC = HH // P   # 8 chunks of the 1024 hidden dims
HP = HC // 2   # 4 DoubleRow h-pair groups
DP = DC // 2   # 3 DoubleRow d-pair groups

SC_RH = 64.0    # rhv / rtb scale
SC_WA = 32.0    # W_a fp8 scale
SC_AT = 128.0   # at fp8 scale
SC_A2 = 32.0    # aw2 fp8 scale
SC_WC = 512.0   # w_c limb scale

_prog_cache = {}


def _build_program(N1, N2P, K, NBLK, ab2_f, cb_f, t_c_f, warm=60):
    import concourse.bass as bass
    import concourse.bacc as bacc
    import concourse.mybir as mybir
    import concourse.tile as tile

    f32 = mybir.dt.float32
    f16 = mybir.dt.float16
    f8 = mybir.dt.float8e4
    Act = mybir.ActivationFunctionType
    Alu = mybir.AluOpType
    Axis = mybir.AxisListType
    DR = mybir.MatmulPerfMode.DoubleRow

    NP = K * N1                 # pairs per block
    NPS = (NP + 15) & ~15       # fp8 plane stride (mult of 16)

    # fp16 blob column offsets: h1t | h2t | wcl
    O1 = 0
    O2 = HC * N1
    OW = O2 + HC * N2P
    W16 = OW + 2 * HC
    # fp8 blob column offsets: wa (4 groups of [2, D]) | aw2 (DP slots of 32)
    OA2 = HP * 2 * D
    W8 = OA2 + DP * 32

    nc = bacc.Bacc(
        "TRN2",
        target_bir_lowering=False,
        debug=False,
        enable_asserts=False,
        num_devices=8,
    )

    b16_d = nc.dram_tensor("b16", [P, W16], f16, kind="ExternalInput").ap()
    b8_d = nc.dram_tensor("b8", [P, W8], f8, kind="ExternalInput").ap()
    bac_d = nc.dram_tensor("bac", [P, DC], f32, kind="ExternalInput").ap()
    vld_d = nc.dram_tensor("vld", [1, NBLK * NPS], f32, kind="ExternalInput").ap()
    y_d = nc.dram_tensor("y", [1, 1], f32, kind="ExternalOutput").ap()

    with tile.TileContext(nc, trace_sim=False) as tc:
        with (
            tc.tile_pool(name="const", bufs=1) as cpool,
            tc.tile_pool(name="work", bufs=1) as work,
            tc.tile_pool(name="ps", bufs=3, space="PSUM") as psp,
            tc.tile_pool(name="psv", bufs=2, space="PSUM") as psv,
        ):
            b16 = cpool.tile([P, W16], f16)
            nc.sync.dma_start(b16[:], b16_d[:])
            vld = cpool.tile([1, NBLK * NPS], f32)
            nc.sync.dma_start(vld[:], vld_d[:])
            bac = cpool.tile([P, DC], f32)
            nc.sync.dma_start(bac[:], bac_d[:])
            b8 = cpool.tile([P, W8], f8)
            nc.scalar.dma_start(b8[:], b8_d[:])

            def h1s(hc):
                return b16[:, O1 + hc * N1:O1 + (hc + 1) * N1]

            def h2s(hc, bi):
                o = O2 + hc * N2P + bi * K
                return b16[:, o:o + K]

            def wcls(j, hc):
                o = OW + j * HC + hc
                return b16[:, o:o + 1]

            def wa3(hp):
                return b8[:, hp * 2 * D:(hp + 1) * 2 * D].rearrange(
                    "p (j d) -> p j d", j=2)

            # PE clock-gate warm-up: dummy accumulation chain on scratch data,
            # no input dependencies, runs during the preamble + input DMAs.
            if warm:
                wsc = cpool.tile([P, 64], f16)
                nc.vector.memset(wsc[:], 0.25)
                wps = psv.tile([64, 64], f32, tag="warm", bufs=1, name="warmps")
                for wi in range(warm):
                    nc.tensor.matmul(
                        wps[:], lhsT=wsc[:, :64], rhs=wsc[:],
                        start=(wi == 0), stop=(wi == warm - 1),
                    )

            Pw_parts = work.tile([1, NBLK], f32, tag="pwp", bufs=1)
            S_parts = work.tile([1, NBLK], f32, tag="sp", bufs=1)

            for bi in range(NBLK):
                # rs = H1[l] + H2[m]  (fp16, [P, HC, K*N1])
                rs = work.tile([P, HC, NP], f16, tag="rs", bufs=3, name=f"rs{bi}")
                for hc in range(HC):
                    nc.vector.tensor_tensor(
                        out=rs[:, hc, :].rearrange("p (k l) -> p k l", k=K),
                        in0=h1s(hc).unsqueeze(1).broadcast_to([P, K, N1]),
                        in1=h2s(hc, bi).unsqueeze(2).broadcast_to([P, K, N1]),
                        op=Alu.add,
                    )
                # rtb = fp8(64*relu(rs)) in DoubleRow layout [P, HC, NPS]
                rtb = work.tile([P, HC, NPS], f8, tag="rtb", bufs=3, name=f"rtb{bi}")
                nc.scalar.activation(
                    rtb[:, :, :NP], rs[:], Act.Relu, scale=SC_RH,
                )
                # rhv = 64*relu(rs) fp16 (value path)
                rhv = work.tile([P, HC, NP], f16, tag="rhv", bufs=3, name=f"rhv{bi}")
                for hc in range(HC):
                    nc.vector.tensor_scalar(
                        out=rhv[:, hc, :], in0=rs[:, hc, :],
                        scalar1=0.0, scalar2=SC_RH,
                        op0=Alu.max, op1=Alu.mult,
                    )

                # v = rhv @ (wcl0 + wcl1) -> pv [1, NP] PSUM (limbs accumulate)
                pv = psv.tile([1, NP], f32, tag="pv", bufs=2, name=f"pv{bi}")
                for j in range(2):
                    for hc in range(HC):
                        nc.tensor.matmul(
                            pv[:],
                            lhsT=wcls(j, hc),
                            rhs=rhv[:, hc, :],
                            start=(j == 0 and hc == 0),
                            stop=(j == 1 and hc == HC - 1),
                        )

                # s = rtb @ W_a  (fp8 DoubleRow), at = fp8(128*relu(s+b_a))
                at = work.tile([P, DC, NPS], f8, tag="at", bufs=2, name=f"at{bi}")
                for dc in range(DC):
                    ps = psp.tile([P, NP], f32, tag="ps", name=f"ps{bi}_{dc}")
                    for hp in range(HP):
                        nc.tensor.matmul(
                            ps[:],
                            lhsT=wa3(hp)[:, :, dc * P:(dc + 1) * P],
                            rhs=rtb[:, 2 * hp:2 * hp + 2, :NP],
                            start=(hp == 0),
                            stop=(hp == HP - 1),
                            perf_mode=DR,
                        )
                    nc.scalar.activation(
                        at[:, dc, :NP], ps[:], Act.Relu,
                        bias=bac[:, dc:dc + 1], scale=SC_AT / (SC_RH * SC_WA),
                    )

                # logit = at @ aw2  ([1, NP] PSUM)
                pl = psv.tile([1, NP], f32, tag="pl", bufs=2, name=f"pl{bi}")
                for dp in range(DP):
                    nc.tensor.matmul(
                        pl[:],
                        lhsT=b8[:, OA2 + dp * 32:OA2 + (dp + 1) * 32].rearrange(
                            "p (j s) -> p j s", j=2)[:, :, 0:1],
                        rhs=at[:, 2 * dp:2 * dp + 2, :NP],
                        start=(dp == 0),
                        stop=(dp == DP - 1),
                        perf_mode=DR,
                    )
                # attn = sigmoid(logit/4096 + ab2)
                attn = work.tile([1, NP], f32, tag="attn", bufs=2, name=f"attn{bi}")
                nc.scalar.activation(
                    attn[:], pl[:], Act.Sigmoid,
                    bias=float(ab2_f), scale=1.0 / (SC_AT * SC_A2),
                )
                # attnm = attn * valid; S_parts[bi] = sum(attnm)
                attnm = work.tile([1, NP], f32, tag="attnm", bufs=2, name=f"attnm{bi}")
                nc.vector.tensor_tensor(
                    out=attnm[:], in0=attn[:],
                    in1=vld[:, bi * NPS:bi * NPS + NP], op=Alu.mult,
                )
                nc.vector.tensor_reduce(
                    out=S_parts[:, bi:bi + 1], in_=attnm[:],
                    axis=Axis.X, op=Alu.add,
                )
                # Pw_parts[bi] = sum(attnm * (v0 + v1))
                vm = work.tile([1, NP], f32, tag="vm", bufs=2, name=f"vm{bi}")
                nc.vector.tensor_tensor(
                    out=vm[:], in0=attnm[:], in1=pv[0:1, :], op=Alu.mult,
                )
                nc.vector.tensor_reduce(
                    out=Pw_parts[:, bi:bi + 1], in_=vm[:],
                    axis=Axis.X, op=Alu.add,
                )

            # ---- final: y = (Pw/(64*512) + S*t_c)/(S+1e-5) + cb ----
            Pw = work.tile([1, 1], f32, tag="pw", bufs=1)
            nc.vector.tensor_reduce(out=Pw[:], in_=Pw_parts[:], axis=Axis.X,
                                    op=Alu.add)
            S = work.tile([1, 1], f32, tag="s", bufs=1)
            nc.vector.tensor_reduce(out=S[:], in_=S_parts[:], axis=Axis.X,
                                    op=Alu.add)
            den = work.tile([1, 1], f32, tag="den", bufs=1)
            nc.vector.tensor_scalar_add(den[:], S[:], 1e-5)
            rden = work.tile([1, 1], f32, tag="rden", bufs=1)
            nc.vector.reciprocal(rden[:], den[:])
            num = work.tile([1, 1], f32, tag="num", bufs=1)
            pw_sc = work.tile([1, 1], f32, tag="pwsc", bufs=1)
            nc.vector.tensor_scalar_mul(pw_sc[:], Pw[:], 1.0 / (SC_RH * SC_WC))
            nc.vector.scalar_tensor_tensor(
                out=num[:], in0=S[:], scalar=float(t_c_f), in1=pw_sc[:],
                op0=Alu.mult, op1=Alu.add,
            )
            y0 = work.tile([1, 1], f32, tag="y0", bufs=1)
            nc.vector.tensor_tensor(out=y0[:], in0=num[:], in1=rden[:],
                                    op=Alu.mult)
            y1 = work.tile([1, 1], f32, tag="y1", bufs=1)
            nc.vector.tensor_scalar_add(y1[:], y0[:], float(cb_f))
            nc.sync.dma_start(y_d[:], y1[:])

    nc.compile()
    return nc


def _prep(x1, x2, mask1, mask2, embed_table, tw1, tb1, tw2, tb2,
          aw1, ab1, aw2, ab2, cw, cb, compact=True):
    """Host-side prep: compaction, weight folding, H matmuls, per-core maps."""
    import ml_dtypes
    f32 = np.float32
    f16 = np.float16
    f8 = ml_dtypes.float8_e4m3fn
    f64 = np.float64

    x1 = np.where(x1 == PAD_ID, 0, x1).astype(np.int32)
    x2 = np.where(x2 == PAD_ID, 0, x2).astype(np.int32)
    w1a = np.ascontiguousarray(tw1[:D]).astype(f64)
    w1b = np.ascontiguousarray(tw1[D:]).astype(f64)
    W_a = (tw2.astype(f64) @ aw1.astype(f64)).astype(f32)
    b_a = (tb2.astype(f64) @ aw1.astype(f64) + ab1.astype(f64)).astype(f32)
    w_c = (tw2.astype(f64) @ cw.astype(f64)).astype(f32).ravel()
    t_c = float(tb2.astype(f64) @ cw.astype(f64).ravel())

    if compact:
        l_lists = [np.nonzero(mask1[b])[0] for b in range(B)]
        m_lists = [np.nonzero(mask2[b])[0] for b in range(B)]
        N1 = max(4, max((len(l) for l in l_lists), default=4))
        N1 = (N1 + 3) & ~3
        N2 = max(1, max((len(m) for m in m_lists), default=1))
    else:
        l_lists = [np.arange(L1) for _ in range(B)]
        m_lists = [np.arange(L2) for _ in range(B)]
        N1, N2 = L1, L2
    K_max = max(1, min(16, 512 // N1))
    NBLK = -(-N2 // K_max)
    K = -(-N2 // NBLK)          # shrink K to just cover N2 in NBLK blocks
    N2P = NBLK * K
    NP = K * N1
    NPS = (NP + 15) & ~15

    O1 = 0
    O2 = HC * N1
    OW = O2 + HC * N2P
    W16 = OW + 2 * HC
    OA2 = HP * 2 * D
    W8 = OA2 + DP * 32

    # fp8 blob: wa groups + aw2 (shared across cores)
    b8_host = np.zeros((P, W8), dtype=f8)
    for hp in range(HP):
        for j in range(2):
            b8_host[:, hp * 2 * D + j * D:(hp * 2 + j + 1) * D] = \
                (SC_WA * W_a[(2 * hp + j) * P:(2 * hp + j + 1) * P, :]).astype(f8)
    a2 = (SC_A2 * aw2.astype(f32).ravel()).reshape(DC, P)  # [dc, p]
    for dp in range(DP):
        for j in range(2):
            b8_host[:, OA2 + dp * 32 + j * 16] = a2[2 * dp + j, :].astype(f8)

    bac_host = np.ascontiguousarray((SC_AT * b_a).reshape(DC, P).T, dtype=f32)
    wc_s = (SC_WC * w_c).astype(f32)
    l0 = wc_s.astype(f16)
    l1 = (wc_s - l0.astype(f32)).astype(f16)

    table = np.asarray(embed_table, dtype=f32)
    in_maps = []
    for b in range(B):
        ll, ml = l_lists[b], m_lists[b]
        n1, n2 = len(ll), len(ml)
        b16_host = np.zeros((P, W16), dtype=f16)
        b16_host[:, OW:OW + HC] = l0.reshape(HC, P).T
        b16_host[:, OW + HC:OW + 2 * HC] = l1.reshape(HC, P).T
        if n1:
            e1 = table[x1[b][ll]].astype(f64)          # [n1, D]
            H1 = (e1 @ w1a).astype(f32)                # [n1, HH]
            h1 = H1.T.reshape(HC, P, n1)               # [hc, p, l]
            b16_host[:, O1:O2].reshape(P, HC, N1)[:, :, :n1] = \
                np.transpose(h1, (1, 0, 2)).astype(f16)
        if n2:
            e2 = table[x2[b][ml]].astype(f64)
            H2 = (e2 @ w1b + tb1.astype(f64)).astype(f32)
            h2 = H2.T.reshape(HC, P, n2)
            b16_host[:, O2:OW].reshape(P, HC, N2P)[:, :, :n2] = \
                np.transpose(h2, (1, 0, 2)).astype(f16)
        vld = np.zeros((NBLK, NPS), dtype=f32)
        if n1 and n2:
            vm = (mask1[b][ll][None, :] != 0) & (mask2[b][ml][:, None] != 0) \
                 & (x1[b][ll][None, :] != x2[b][ml][:, None])   # [n2, n1]
            grid = np.zeros((N2P, N1), dtype=f32)
            grid[:n2, :n1] = vm.astype(f32)
            vld[:, :NP] = grid.reshape(NBLK, K * N1)
        in_maps.append({
            "b16": b16_host,
            "b8": b8_host,
            "bac": bac_host,
            "vld": vld.reshape(1, NBLK * NPS),
        })
    ab2_f = float(np.asarray(ab2).ravel()[0])
    cb_f = float(np.asarray(cb).ravel()[0])
    return (N1, N2P, K, NBLK, ab2_f, cb_f, t_c), in_maps


def kernel(x1, x2, mask1, mask2, embed_table, tw1, tb1, tw2, tb2,
           aw1, ab1, aw2, ab2, cw, cb):
    from concourse import bass_utils

    key_args, in_maps = _prep(
        x1, x2, mask1, mask2, embed_table, tw1, tb1, tw2, tb2,
        aw1, ab1, aw2, ab2, cw, cb)

    if key_args not in _prog_cache:
        _prog_cache[key_args] = _build_program(*key_args)
    nc = _prog_cache[key_args]

    res = bass_utils.run_bass_kernel_spmd(nc, in_maps, core_ids=list(range(8)))
    y = np.stack([res.results[i]["y"].reshape(()) for i in range(B)])
    return y.reshape(B, 1).astype(np.float32)


# revision 40
# speedup vs baseline: 2.2672x; 1.0173x over previous
"""Trainium2 Bass kernel for nn_CrossAttentionModel (cross-attention pooling).

Strategy (v3)
-------------
Data-parallel over batch: core i handles batch item i (B=8, 8 cores, no
collectives).  Host folds the weight chain and precomputes the tiny
per-sequence H matrices; the device runs the O(NT*H) pair grid, which is
>95% of the FLOPs:

    H1 = e1 @ w1a              (host, [n1,H])
    H2 = e2 @ w1b + tb1        (host, [n2,H])
    rs    = H1[l] + H2[m]                      DVE  (fp16)
    rhv   = relu(rs) * 64                      DVE  (fp16, value path)
    rtb   = fp8(relu(rs) * 64)                 ACT  (one fused pass)
    s     = rtb @ (32*W_a.fp8)                 PE   fp8 DoubleRow (2 col/cyc)
    at    = fp8(128*relu(s + b_a))             ACT
    logit = at @ (32*aw2.fp8)                  PE   fp8 DoubleRow
    attn  = sigmoid(logit/4096 + ab2)          ACT
    attnm = attn * valid                       DVE  (+ TR -> S)
    v     = rhv @ (512*w_c as 2 fp16 limbs)    PE   (one [128,2] stationary)
    Pw   += sum(attnm * (v0+v1))               DVE
    y     = (Pw/(64*512) + S*t_c)/(S+1e-5)+cb

W_a = tw2 @ aw1 folds the trans-MLP second layer into the attn MLP
(emb @ aw1 = rh @ W_a + tb2@aw1), and w_c = tw2 @ cw projects the pooled
value so only the scalar per-pair projection v_p = rh_p . w_c is needed
(pooled @ cw = (sum attn*rh) @ w_c / denom + ...).  The fp8 attention path
is safe because logits are tiny (sigmoid ~ 0.5 + logit/4); the value path
needs w_c kept near-exact (shared quantization noise does not average out
over pairs), hence the two-limb fp16 split of w_c on the PE.

HW notes: tensor_tensor_reduce on single-partition rows crashes the TRN2
exec unit (use TT+TR); DoubleRow lhsT plane strides must be 16B-aligned;
a dummy-matmul chain at program start warms the PE clock gate during the
runtime preamble + input DMAs.
"""

import numpy as np

B, L1, L2, D, HH, V = 8, 64, 64, 768, 1024, 50257
PAD_ID = 50257
P = 128
DC = D // P    # 6 chunks of the 768 attn dims
HC = HH // P   # 8 chunks of the 1024 hidden dims
HP = HC // 2   # 4 DoubleRow h-pair groups
DP = DC // 2   # 3 DoubleRow d-pair groups

SC_RH = 64.0    # rhv / rtb scale
SC_WA = 32.0    # W_a fp8 scale
SC_AT = 128.0   # at fp8 scale
SC_A2 = 32.0    # aw2 fp8 scale
SC_WC = 512.0   # w_c limb scale

_prog_cache = {}


def _build_program(N1, N2P, K, NBLK, ab2_f, cb_f, t_c_f, warm=130):
    import concourse.bass as bass
    import concourse.bacc as bacc
    import concourse.mybir as mybir
    import concourse.tile as tile

    f32 = mybir.dt.float32
    f16 = mybir.dt.float16
    f8 = mybir.dt.float8e4
    Act = mybir.ActivationFunctionType
    Alu = mybir.AluOpType
    Axis = mybir.AxisListType
    DR = mybir.MatmulPerfMode.DoubleRow

    NP = K * N1                 # pairs per block
    NPS = (NP + 15) & ~15       # fp8 plane stride (mult of 16)

    # fp16 blob column offsets: h1t | h2t | wcl
    O1 = 0
    O2 = HC * N1
    OW = O2 + HC * N2P
    W16 = OW + 2 * HC
    # fp8 blob column offsets: wa (4 groups of [2, D]) | aw2 (DP slots of 32)
    OA2 = HP * 2 * D
    W8 = OA2 + DP * 32

    nc = bacc.Bacc(
        "TRN2",
        target_bir_lowering=False,
        debug=False,
        enable_asserts=False,
        num_devices=8,
    )

    b16_d = nc.dram_tensor("b16", [P, W16], f16, kind="ExternalInput").ap()
    b8_d = nc.dram_tensor("b8", [P, W8], f8, kind="ExternalInput").ap()
    bac_d = nc.dram_tensor("bac", [P, DC], f32, kind="ExternalInput").ap()
    vld_d = nc.dram_tensor("vld", [1, NBLK * NPS], f32, kind="ExternalInput").ap()
    y_d = nc.dram_tensor("y", [1, 1], f32, kind="ExternalOutput").ap()

    with tile.TileContext(nc, trace_sim=False) as tc:
        with (
            tc.tile_pool(name="const", bufs=1) as cpool,
            tc.tile_pool(name="work", bufs=1) as work,
            tc.tile_pool(name="ps", bufs=3, space="PSUM") as psp,
            tc.tile_pool(name="psv", bufs=2, space="PSUM") as psv,
        ):
            b16 = cpool.tile([P, W16], f16)
            nc.sync.dma_start(b16[:], b16_d[:])
            vld = cpool.tile([1, NBLK * NPS], f32)
            nc.sync.dma_start(vld[:], vld_d[:])
            bac = cpool.tile([P, DC], f32)
            nc.sync.dma_start(bac[:], bac_d[:])
            b8 = cpool.tile([P, W8], f8)
            nc.scalar.dma_start(b8[:], b8_d[:])

            def h1s(hc):
                return b16[:, O1 + hc * N1:O1 + (hc + 1) * N1]

            def h2s(hc, bi):
                o = O2 + hc * N2P + bi * K
                return b16[:, o:o + K]

            def wcls(j, hc):
                o = OW + j * HC + hc
                return b16[:, o:o + 1]

            def wa3(hp):
                return b8[:, hp * 2 * D:(hp + 1) * 2 * D].rearrange(
                    "p (j d) -> p j d", j=2)

            # PE clock-gate warm-up: dummy accumulation chain on scratch data,
            # no input dependencies, runs during the preamble + input DMAs.
            if warm:
                wsc = cpool.tile([P, 64], f16)
                nc.vector.memset(wsc[:], 0.25)
                wps = psv.tile([64, 64], f32, tag="warm", bufs=1, name="warmps")
                for wi in range(warm):
                    nc.tensor.matmul(
                        wps[:], lhsT=wsc[:, :64], rhs=wsc[:],
                        start=(wi == 0), stop=(wi == warm - 1),
                    )

            Pw_parts = work.tile([1, NBLK], f32, tag="pwp", bufs=1)
            S_parts = work.tile([1, NBLK], f32, tag="sp", bufs=1)

            for bi in range(NBLK):
                # rs = H1[l] + H2[m]  (fp16, [P, HC, K*N1])
                rs = work.tile([P, HC, NP], f16, tag="rs", bufs=3, name=f"rs{bi}")
                for hc in range(HC):
                    nc.vector.tensor_tensor(
                        out=rs[:, hc, :].rearrange("p (k l) -> p k l", k=K),
                        in0=h1s(hc).unsqueeze(1).broadcast_to([P, K, N1]),
                        in1=h2s(hc, bi).unsqueeze(2).broadcast_to([P, K, N1]),
                        op=Alu.add,
                    )
                # rtb = fp8(64*relu(rs)) in DoubleRow layout [P, HC, NPS]
                # (two halves so the first s-matmuls can start earlier)
                rtb = work.tile([P, HC, NPS], f8, tag="rtb", bufs=3, name=f"rtb{bi}")
                hh = HC // 2
                nc.scalar.activation(
                    rtb[:, :hh, :NP], rs[:, :hh, :], Act.Relu, scale=SC_RH,
                )
                nc.scalar.activation(
                    rtb[:, hh:, :NP], rs[:, hh:, :], Act.Relu, scale=SC_RH,
                )
                # rhv = 64*relu(rs) fp16 (value path)
                rhv = work.tile([P, HC, NP], f16, tag="rhv", bufs=3, name=f"rhv{bi}")
                for hc in range(HC):
                    nc.vector.tensor_scalar(
                        out=rhv[:, hc, :], in0=rs[:, hc, :],
                        scalar1=0.0, scalar2=SC_RH,
                        op0=Alu.max, op1=Alu.mult,
                    )

                # v = rhv @ (wcl0 + wcl1) -> pv [1, NP] PSUM (limbs accumulate)
                pv = psv.tile([1, NP], f32, tag="pv", bufs=2, name=f"pv{bi}")
                for j in range(2):
                    for hc in range(HC):
                        nc.tensor.matmul(
                            pv[:],
                            lhsT=wcls(j, hc),
                            rhs=rhv[:, hc, :],
                            start=(j == 0 and hc == 0),
                            stop=(j == 1 and hc == HC - 1),
                        )

                # s = rtb @ W_a  (fp8 DoubleRow), at = fp8(128*relu(s+b_a))
                at = work.tile([P, DC, NPS], f8, tag="at", bufs=2, name=f"at{bi}")
                for dc in range(DC):
                    ps = psp.tile([P, NP], f32, tag="ps", name=f"ps{bi}_{dc}")
                    for hp in range(HP):
                        nc.tensor.matmul(
                            ps[:],
                            lhsT=wa3(hp)[:, :, dc * P:(dc + 1) * P],
                            rhs=rtb[:, 2 * hp:2 * hp + 2, :NP],
                            start=(hp == 0),
                            stop=(hp == HP - 1),
                            perf_mode=DR,
                        )
                    nc.scalar.activation(
                        at[:, dc, :NP], ps[:], Act.Relu,
                        bias=bac[:, dc:dc + 1], scale=SC_AT / (SC_RH * SC_WA),
                    )

                # logit = at @ aw2  ([1, NP] PSUM)
                pl = psv.tile([1, NP], f32, tag="pl", bufs=2, name=f"pl{bi}")
                for dp in range(DP):
                    nc.tensor.matmul(
                        pl[:],
                        lhsT=b8[:, OA2 + dp * 32:OA2 + (dp + 1) * 32].rearrange(
                            "p (j s) -> p j s", j=2)[:, :, 0:1],
                        rhs=at[:, 2 * dp:2 * dp + 2, :NP],
                        start=(dp == 0),
                        stop=(dp == DP - 1),
                        perf_mode=DR,
                    )
                # attn = sigmoid(logit/4096 + ab2)
                attn = work.tile([1, NP], f32, tag="attn", bufs=2, name=f"attn{bi}")
                nc.scalar.activation(
                    attn[:], pl[:], Act.Sigmoid,
                    bias=float(ab2_f), scale=1.0 / (SC_AT * SC_A2),
                )
                # attnm = attn * valid; S_parts[bi] = sum(attnm)
                attnm = work.tile([1, NP], f32, tag="attnm", bufs=2, name=f"attnm{bi}")
                nc.vector.tensor_tensor(
                    out=attnm[:], in0=attn[:],
                    in1=vld[:, bi * NPS:bi * NPS + NP], op=Alu.mult,
                )
                nc.vector.tensor_reduce(
                    out=S_parts[:, bi:bi + 1], in_=attnm[:],
                    axis=Axis.X, op=Alu.add,
                )
                # Pw_parts[bi] = sum(attnm * (v0 + v1))
                vm = work.tile([1, NP], f32, tag="vm", bufs=2, name=f"vm{bi}")
                nc.vector.tensor_tensor(
                    out=vm[:], in0=attnm[:], in1=pv[0:1, :], op=Alu.mult,
                )
                nc.vector.tensor_reduce(
                    out=Pw_parts[:, bi:bi + 1], in_=vm[:],
                    axis=Axis.X, op=Alu.add,
                )

            # ---- final: y = (Pw/(64*512) + S*t_c)/(S+1e-5) + cb ----
            Pw = work.tile([1, 1], f32, tag="pw", bufs=1)
            nc.vector.tensor_reduce(out=Pw[:], in_=Pw_parts[:], axis=Axis.X,
                                    op=Alu.add)
            S = work.tile([1, 1], f32, tag="s", bufs=1)
            nc.vector.tensor_reduce(out=S[:], in_=S_parts[:], axis=Axis.X,
                                    op=Alu.add)
            den = work.tile([1, 1], f32, tag="den", bufs=1)
            nc.vector.tensor_scalar_add(den[:], S[:], 1e-5)
            rden = work.tile([1, 1], f32, tag="rden", bufs=1)
            nc.vector.reciprocal(rden[:], den[:])
            num = work.tile([1, 1], f32, tag="num", bufs=1)
            pw_sc = work.tile([1, 1], f32, tag="pwsc", bufs=1)
            nc.vector.tensor_scalar_mul(pw_sc[:], Pw[:], 1.0 / (SC_RH * SC_WC))
            nc.vector.scalar_tensor_tensor(
                out=num[:], in0=S[:], scalar=float(t_c_f), in1=pw_sc[:],
                op0=Alu.mult, op1=Alu.add,
            )
            y0 = work.tile([1, 1], f32, tag="y0", bufs=1)
            nc.vector.tensor_tensor(out=y0[:], in0=num[:], in1=rden[:],
                                    op=Alu.mult)
            y1 = work.tile([1, 1], f32, tag="y1", bufs=1)
            nc.vector.tensor_scalar_add(y1[:], y0[:], float(cb_f))
            nc.sync.dma_start(y_d[:], y1[:])

    nc.compile()
    return nc


def _prep(x1, x2, mask1, mask2, embed_table, tw1, tb1, tw2, tb2,
          aw1, ab1, aw2, ab2, cw, cb, compact=True):
    """Host-side prep: compaction, weight folding, H matmuls, per-core maps."""
    import ml_dtypes
    f32 = np.float32
    f16 = np.float16
    f8 = ml_dtypes.float8_e4m3fn
    f64 = np.float64

    x1 = np.where(x1 == PAD_ID, 0, x1).astype(np.int32)
    x2 = np.where(x2 == PAD_ID, 0, x2).astype(np.int32)
    w1a = np.ascontiguousarray(tw1[:D]).astype(f64)
    w1b = np.ascontiguousarray(tw1[D:]).astype(f64)
    W_a = (tw2.astype(f64) @ aw1.astype(f64)).astype(f32)
    b_a = (tb2.astype(f64) @ aw1.astype(f64) + ab1.astype(f64)).astype(f32)
    w_c = (tw2.astype(f64) @ cw.astype(f64)).astype(f32).ravel()
    t_c = float(tb2.astype(f64) @ cw.astype(f64).ravel())

    if compact:
        l_lists = [np.nonzero(mask1[b])[0] for b in range(B)]
        m_lists = [np.nonzero(mask2[b])[0] for b in range(B)]
        N1 = max(4, max((len(l) for l in l_lists), default=4))
        N1 = (N1 + 3) & ~3
        N2 = max(1, max((len(m) for m in m_lists), default=1))
    else:
        l_lists = [np.arange(L1) for _ in range(B)]
        m_lists = [np.arange(L2) for _ in range(B)]
        N1, N2 = L1, L2
    K_max = max(1, min(16, 512 // N1))
    NBLK = -(-N2 // K_max)
    K = -(-N2 // NBLK)          # shrink K to just cover N2 in NBLK blocks
    N2P = NBLK * K
    NP = K * N1
    NPS = (NP + 15) & ~15

    O1 = 0
    O2 = HC * N1
    OW = O2 + HC * N2P
    W16 = OW + 2 * HC
    OA2 = HP * 2 * D
    W8 = OA2 + DP * 32

    # fp8 blob: wa groups + aw2 (shared across cores)
    b8_host = np.zeros((P, W8), dtype=f8)
    for hp in range(HP):
        for j in range(2):
            b8_host[:, hp * 2 * D + j * D:(hp * 2 + j + 1) * D] = \
                (SC_WA * W_a[(2 * hp + j) * P:(2 * hp + j + 1) * P, :]).astype(f8)
    a2 = (SC_A2 * aw2.astype(f32).ravel()).reshape(DC, P)  # [dc, p]
    for dp in range(DP):
        for j in range(2):
            b8_host[:, OA2 + dp * 32 + j * 16] = a2[2 * dp + j, :].astype(f8)

    bac_host = np.ascontiguousarray((SC_AT * b_a).reshape(DC, P).T, dtype=f32)
    wc_s = (SC_WC * w_c).astype(f32)
    l0 = wc_s.astype(f16)
    l1 = (wc_s - l0.astype(f32)).astype(f16)

    table = np.asarray(embed_table, dtype=f32)
    in_maps = []
    for b in range(B):
        ll, ml = l_lists[b], m_lists[b]
        n1, n2 = len(ll), len(ml)
        b16_host = np.zeros((P, W16), dtype=f16)
        b16_host[:, OW:OW + HC] = l0.reshape(HC, P).T
        b16_host[:, OW + HC:OW + 2 * HC] = l1.reshape(HC, P).T
        if n1:
            e1 = table[x1[b][ll]].astype(f64)          # [n1, D]
            H1 = (e1 @ w1a).astype(f32)                # [n1, HH]
            h1 = H1.T.reshape(HC, P, n1)               # [hc, p, l]
            b16_host[:, O1:O2].reshape(P, HC, N1)[:, :, :n1] = \
                np.transpose(h1, (1, 0, 2)).astype(f16)
        if n2:
            e2 = table[x2[b][ml]].astype(f64)
            H2 = (e2 @ w1b + tb1.astype(f64)).astype(f32)
            h2 = H2.T.reshape(HC, P, n2)
            b16_host[:, O2:OW].reshape(P, HC, N2P)[:, :, :n2] = \
                np.transpose(h2, (1, 0, 2)).astype(f16)
        vld = np.zeros((NBLK, NPS), dtype=f32)
        if n1 and n2:
            vm = (mask1[b][ll][None, :] != 0) & (mask2[b][ml][:, None] != 0) \
                 & (x1[b][ll][None, :] != x2[b][ml][:, None])   # [n2, n1]
            grid = np.zeros((N2P, N1), dtype=f32)
            grid[:n2, :n1] = vm.astype(f32)
            vld[:, :NP] = grid.reshape(NBLK, K * N1)
        in_maps.append({
            "b16": b16_host,
            "b8": b8_host,
            "bac": bac_host,
            "vld": vld.reshape(1, NBLK * NPS),
        })
    ab2_f = float(np.asarray(ab2).ravel()[0])
    cb_f = float(np.asarray(cb).ravel()[0])
    return (N1, N2P, K, NBLK, ab2_f, cb_f, t_c), in_maps


def kernel(x1, x2, mask1, mask2, embed_table, tw1, tb1, tw2, tb2,
           aw1, ab1, aw2, ab2, cw, cb):
    from concourse import bass_utils

    key_args, in_maps = _prep(
        x1, x2, mask1, mask2, embed_table, tw1, tb1, tw2, tb2,
        aw1, ab1, aw2, ab2, cw, cb)

    if key_args not in _prog_cache:
        _prog_cache[key_args] = _build_program(*key_args)
    nc = _prog_cache[key_args]

    res = bass_utils.run_bass_kernel_spmd(nc, in_maps, core_ids=list(range(8)))
    y = np.stack([res.results[i]["y"].reshape(()) for i in range(B)])
    return y.reshape(B, 1).astype(np.float32)


# revision 42
# speedup vs baseline: 2.2685x; 1.0006x over previous
"""Trainium2 Bass kernel for nn_CrossAttentionModel (cross-attention pooling).

Strategy (v3)
-------------
Data-parallel over batch: core i handles batch item i (B=8, 8 cores, no
collectives).  Host folds the weight chain and precomputes the tiny
per-sequence H matrices; the device runs the O(NT*H) pair grid, which is
>95% of the FLOPs:

    H1 = e1 @ w1a              (host, [n1,H])
    H2 = e2 @ w1b + tb1        (host, [n2,H])
    rs    = H1[l] + H2[m]                      DVE  (fp16)
    rhv   = relu(rs) * 64                      DVE  (fp16, value path)
    rtb   = fp8(relu(rs) * 64)                 ACT  (one fused pass)
    s     = rtb @ (32*W_a.fp8)                 PE   fp8 DoubleRow (2 col/cyc)
    at    = fp8(128*relu(s + b_a))             ACT
    logit = at @ (32*aw2.fp8)                  PE   fp8 DoubleRow
    attn  = sigmoid(logit/4096 + ab2)          ACT
    attnm = attn * valid                       DVE  (+ TR -> S)
    v     = rhv @ (512*w_c as 2 fp16 limbs)    PE   (one [128,2] stationary)
    Pw   += sum(attnm * (v0+v1))               DVE
    y     = (Pw/(64*512) + S*t_c)/(S+1e-5)+cb

W_a = tw2 @ aw1 folds the trans-MLP second layer into the attn MLP
(emb @ aw1 = rh @ W_a + tb2@aw1), and w_c = tw2 @ cw projects the pooled
value so only the scalar per-pair projection v_p = rh_p . w_c is needed
(pooled @ cw = (sum attn*rh) @ w_c / denom + ...).  The fp8 attention path
is safe because logits are tiny (sigmoid ~ 0.5 + logit/4); the value path
needs w_c kept near-exact (shared quantization noise does not average out
over pairs), hence the two-limb fp16 split of w_c on the PE.

HW notes: tensor_tensor_reduce on single-partition rows crashes the TRN2
exec unit (use TT+TR); DoubleRow lhsT plane strides must be 16B-aligned;
a dummy-matmul chain at program start warms the PE clock gate during the
runtime preamble + input DMAs.
"""

import numpy as np

B, L1, L2, D, HH, V = 8, 64, 64, 768, 1024, 50257
PAD_ID = 50257
P = 128
DC = D // P    # 6 chunks of the 768 attn dims
HC = HH // P   # 8 chunks of the 1024 hidden dims
HP = HC // 2   # 4 DoubleRow h-pair groups
DP = DC // 2   # 3 DoubleRow d-pair groups

SC_RH = 64.0    # rhv / rtb scale
SC_WA = 32.0    # W_a fp8 scale
SC_AT = 128.0   # at fp8 scale
SC_A2 = 32.0    # aw2 fp8 scale
SC_WC = 512.0   # w_c limb scale

_prog_cache = {}


def _build_program(N1, N2P, K, NBLK, ab2_f, cb_f, t_c_f, warm=130):
    import concourse.bass as bass
    import concourse.bacc as bacc
    import concourse.mybir as mybir
    import concourse.tile as tile

    f32 = mybir.dt.float32
    f16 = mybir.dt.float16
    f8 = mybir.dt.float8e4
    Act = mybir.ActivationFunctionType
    Alu = mybir.AluOpType
    Axis = mybir.AxisListType
    DR = mybir.MatmulPerfMode.DoubleRow

    NP = K * N1                 # pairs per block
    NPS = (NP + 15) & ~15       # fp8 plane stride (mult of 16)

    # fp16 blob column offsets: h1t | h2t | wcl
    O1 = 0
    O2 = HC * N1
    OW = O2 + HC * N2P
    W16 = OW + 2 * HC
    # fp8 blob column offsets: wa (4 groups of [2, D]) | aw2 (DP slots of 32)
    OA2 = HP * 2 * D
    W8 = OA2 + DP * 32

    nc = bacc.Bacc(
        "TRN2",
        target_bir_lowering=False,
        debug=False,
        enable_asserts=False,
        num_devices=8,
    )

    b16_d = nc.dram_tensor("b16", [P, W16], f16, kind="ExternalInput").ap()
    b8_d = nc.dram_tensor("b8", [P, W8], f8, kind="ExternalInput").ap()
    bac_d = nc.dram_tensor("bac", [P, DC], f32, kind="ExternalInput").ap()
    vld_d = nc.dram_tensor("vld", [1, NBLK * NPS], f32, kind="ExternalInput").ap()
    y_d = nc.dram_tensor("y", [1, 1], f32, kind="ExternalOutput").ap()

    with tile.TileContext(nc, trace_sim=False) as tc:
        with (
            tc.tile_pool(name="const", bufs=1) as cpool,
            tc.tile_pool(name="work", bufs=1) as work,
            tc.tile_pool(name="ps", bufs=3, space="PSUM") as psp,
            tc.tile_pool(name="psv", bufs=2, space="PSUM") as psv,
        ):
            b16 = cpool.tile([P, W16], f16)
            nc.sync.dma_start(b16[:], b16_d[:])
            vld = cpool.tile([1, NBLK * NPS], f32)
            nc.sync.dma_start(vld[:], vld_d[:])
            bac = cpool.tile([P, DC], f32)
            nc.sync.dma_start(bac[:], bac_d[:])
            b8 = cpool.tile([P, W8], f8)
            nc.scalar.dma_start(b8[:], b8_d[:])

            def h1s(hc):
                return b16[:, O1 + hc * N1:O1 + (hc + 1) * N1]

            def h2s(hc, bi):
                o = O2 + hc * N2P + bi * K
                return b16[:, o:o + K]

            def wcls(j, hc):
                o = OW + j * HC + hc
                return b16[:, o:o + 1]

            def wa3(hp):
                return b8[:, hp * 2 * D:(hp + 1) * 2 * D].rearrange(
                    "p (j d) -> p j d", j=2)

            # PE clock-gate warm-up: dummy accumulation chain on scratch data,
            # no input dependencies, runs during the preamble + input DMAs.
            if warm:
                wsc = cpool.tile([P, 64], f16)
                nc.vector.memset(wsc[:], 0.25)
                wps = psv.tile([64, 64], f32, tag="warm", bufs=1, name="warmps")
                for wi in range(warm):
                    nc.tensor.matmul(
                        wps[:], lhsT=wsc[:, :64], rhs=wsc[:],
                        start=(wi == 0), stop=(wi == warm - 1),
                    )

            Pw_parts = work.tile([1, NBLK], f32, tag="pwp", bufs=1)
            S_parts = work.tile([1, NBLK], f32, tag="sp", bufs=1)

            for bi in range(NBLK):
                # rs = H1[l] + H2[m]  (fp16, [P, HC, K*N1])
                rs = work.tile([P, HC, NP], f16, tag="rs", bufs=3, name=f"rs{bi}")
                rhv = work.tile([P, HC, NP], f16, tag="rhv", bufs=3, name=f"rhv{bi}")
                for hc in range(HC):
                    nc.vector.tensor_tensor(
                        out=rs[:, hc, :].rearrange("p (k l) -> p k l", k=K),
                        in0=h1s(hc).unsqueeze(1).broadcast_to([P, K, N1]),
                        in1=h2s(hc, bi).unsqueeze(2).broadcast_to([P, K, N1]),
                        op=Alu.add,
                    )
                    # rhv right after its rs chunk so the v-matmuls start early
                    nc.vector.tensor_scalar(
                        out=rhv[:, hc, :], in0=rs[:, hc, :],
                        scalar1=0.0, scalar2=SC_RH,
                        op0=Alu.max, op1=Alu.mult,
                    )
                # rtb = fp8(64*relu(rs)) in DoubleRow layout [P, HC, NPS]
                # (two halves so the first s-matmuls can start earlier)
                rtb = work.tile([P, HC, NPS], f8, tag="rtb", bufs=3, name=f"rtb{bi}")
                hh = HC // 2
                nc.scalar.activation(
                    rtb[:, :hh, :NP], rs[:, :hh, :], Act.Relu, scale=SC_RH,
                )
                nc.scalar.activation(
                    rtb[:, hh:, :NP], rs[:, hh:, :], Act.Relu, scale=SC_RH,
                )

                # v = rhv @ (wcl0 + wcl1) -> pv [1, NP] PSUM (limbs accumulate)
                pv = psv.tile([1, NP], f32, tag="pv", bufs=2, name=f"pv{bi}")
                for j in range(2):
                    for hc in range(HC):
                        nc.tensor.matmul(
                            pv[:],
                            lhsT=wcls(j, hc),
                            rhs=rhv[:, hc, :],
                            start=(j == 0 and hc == 0),
                            stop=(j == 1 and hc == HC - 1),
                        )

                # s = rtb @ W_a  (fp8 DoubleRow), at = fp8(128*relu(s+b_a))
                at = work.tile([P, DC, NPS], f8, tag="at", bufs=2, name=f"at{bi}")
                for dc in range(DC):
                    ps = psp.tile([P, NP], f32, tag="ps", name=f"ps{bi}_{dc}")
                    for hp in range(HP):
                        nc.tensor.matmul(
                            ps[:],
                            lhsT=wa3(hp)[:, :, dc * P:(dc + 1) * P],
                            rhs=rtb[:, 2 * hp:2 * hp + 2, :NP],
                            start=(hp == 0),
                            stop=(hp == HP - 1),
                            perf_mode=DR,
                        )
                    nc.scalar.activation(
                        at[:, dc, :NP], ps[:], Act.Relu,
                        bias=bac[:, dc:dc + 1], scale=SC_AT / (SC_RH * SC_WA),
                    )

                # logit = at @ aw2  ([1, NP] PSUM)
                pl = psv.tile([1, NP], f32, tag="pl", bufs=2, name=f"pl{bi}")
                for dp in range(DP):
                    nc.tensor.matmul(
                        pl[:],
                        lhsT=b8[:, OA2 + dp * 32:OA2 + (dp + 1) * 32].rearrange(
                            "p (j s) -> p j s", j=2)[:, :, 0:1],
                        rhs=at[:, 2 * dp:2 * dp + 2, :NP],
                        start=(dp == 0),
                        stop=(dp == DP - 1),
                        perf_mode=DR,
                    )
                # attn = sigmoid(logit/4096 + ab2)
                attn = work.tile([1, NP], f32, tag="attn", bufs=2, name=f"attn{bi}")
                nc.scalar.activation(
                    attn[:], pl[:], Act.Sigmoid,
                    bias=float(ab2_f), scale=1.0 / (SC_AT * SC_A2),
                )
                # attnm = attn * valid; S_parts[bi] = sum(attnm)
                attnm = work.tile([1, NP], f32, tag="attnm", bufs=2, name=f"attnm{bi}")
                nc.vector.tensor_tensor(
                    out=attnm[:], in0=attn[:],
                    in1=vld[:, bi * NPS:bi * NPS + NP], op=Alu.mult,
                )
                nc.vector.tensor_reduce(
                    out=S_parts[:, bi:bi + 1], in_=attnm[:],
                    axis=Axis.X, op=Alu.add,
                )
                # Pw_parts[bi] = sum(attnm * (v0 + v1))
                vm = work.tile([1, NP], f32, tag="vm", bufs=2, name=f"vm{bi}")
                nc.vector.tensor_tensor(
                    out=vm[:], in0=attnm[:], in1=pv[0:1, :], op=Alu.mult,
                )
                nc.vector.tensor_reduce(
                    out=Pw_parts[:, bi:bi + 1], in_=vm[:],
                    axis=Axis.X, op=Alu.add,
                )

            # ---- final: y = (Pw/(64*512) + S*t_c)/(S+1e-5) + cb ----
            Pw = work.tile([1, 1], f32, tag="pw", bufs=1)
            nc.vector.tensor_reduce(out=Pw[:], in_=Pw_parts[:], axis=Axis.X,
                                    op=Alu.add)
            S = work.tile([1, 1], f32, tag="s", bufs=1)
            nc.vector.tensor_reduce(out=S[:], in_=S_parts[:], axis=Axis.X,
                                    op=Alu.add)
            den = work.tile([1, 1], f32, tag="den", bufs=1)
            nc.vector.tensor_scalar_add(den[:], S[:], 1e-5)
            rden = work.tile([1, 1], f32, tag="rden", bufs=1)
            nc.vector.reciprocal(rden[:], den[:])
            num = work.tile([1, 1], f32, tag="num", bufs=1)
            pw_sc = work.tile([1, 1], f32, tag="pwsc", bufs=1)
            nc.vector.tensor_scalar_mul(pw_sc[:], Pw[:], 1.0 / (SC_RH * SC_WC))
            nc.vector.scalar_tensor_tensor(
                out=num[:], in0=S[:], scalar=float(t_c_f), in1=pw_sc[:],
                op0=Alu.mult, op1=Alu.add,
            )
            y0 = work.tile([1, 1], f32, tag="y0", bufs=1)
            nc.vector.tensor_tensor(out=y0[:], in0=num[:], in1=rden[:],
                                    op=Alu.mult)
            y1 = work.tile([1, 1], f32, tag="y1", bufs=1)
            nc.vector.tensor_scalar_add(y1[:], y0[:], float(cb_f))
            nc.sync.dma_start(y_d[:], y1[:])

    nc.compile()
    return nc


def _prep(x1, x2, mask1, mask2, embed_table, tw1, tb1, tw2, tb2,
          aw1, ab1, aw2, ab2, cw, cb, compact=True):
    """Host-side prep: compaction, weight folding, H matmuls, per-core maps."""
    import ml_dtypes
    f32 = np.float32
    f16 = np.float16
    f8 = ml_dtypes.float8_e4m3fn
    f64 = np.float64

    x1 = np.where(x1 == PAD_ID, 0, x1).astype(np.int32)
    x2 = np.where(x2 == PAD_ID, 0, x2).astype(np.int32)
    w1a = np.ascontiguousarray(tw1[:D]).astype(f64)
    w1b = np.ascontiguousarray(tw1[D:]).astype(f64)
    W_a = (tw2.astype(f64) @ aw1.astype(f64)).astype(f32)
    b_a = (tb2.astype(f64) @ aw1.astype(f64) + ab1.astype(f64)).astype(f32)
    w_c = (tw2.astype(f64) @ cw.astype(f64)).astype(f32).ravel()
    t_c = float(tb2.astype(f64) @ cw.astype(f64).ravel())

    if compact:
        l_lists = [np.nonzero(mask1[b])[0] for b in range(B)]
        m_lists = [np.nonzero(mask2[b])[0] for b in range(B)]
        N1 = max(4, max((len(l) for l in l_lists), default=4))
        N1 = (N1 + 3) & ~3
        N2 = max(1, max((len(m) for m in m_lists), default=1))
    else:
        l_lists = [np.arange(L1) for _ in range(B)]
        m_lists = [np.arange(L2) for _ in range(B)]
        N1, N2 = L1, L2
    K_max = max(1, min(16, 512 // N1))
    NBLK = -(-N2 // K_max)
    K = -(-N2 // NBLK)          # shrink K to just cover N2 in NBLK blocks
    N2P = NBLK * K
    NP = K * N1
    NPS = (NP + 15) & ~15

    O1 = 0
    O2 = HC * N1
    OW = O2 + HC * N2P
    W16 = OW + 2 * HC
    OA2 = HP * 2 * D
    W8 = OA2 + DP * 32

    # fp8 blob: wa groups + aw2 (shared across cores)
    b8_host = np.zeros((P, W8), dtype=f8)
    for hp in range(HP):
        for j in range(2):
            b8_host[:, hp * 2 * D + j * D:(hp * 2 + j + 1) * D] = \
                (SC_WA * W_a[(2 * hp + j) * P:(2 * hp + j + 1) * P, :]).astype(f8)
    a2 = (SC_A2 * aw2.astype(f32).ravel()).reshape(DC, P)  # [dc, p]
    for dp in range(DP):
        for j in range(2):
            b8_host[:, OA2 + dp * 32 + j * 16] = a2[2 * dp + j, :].astype(f8)

    bac_host = np.ascontiguousarray((SC_AT * b_a).reshape(DC, P).T, dtype=f32)
    wc_s = (SC_WC * w_c).astype(f32)
    l0 = wc_s.astype(f16)
    l1 = (wc_s - l0.astype(f32)).astype(f16)

    table = np.asarray(embed_table, dtype=f32)
    in_maps = []
    for b in range(B):
        ll, ml = l_lists[b], m_lists[b]
        n1, n2 = len(ll), len(ml)
        b16_host = np.zeros((P, W16), dtype=f16)
        b16_host[:, OW:OW + HC] = l0.reshape(HC, P).T
        b16_host[:, OW + HC:OW + 2 * HC] = l1.reshape(HC, P).T
        if n1:
            e1 = table[x1[b][ll]].astype(f64)          # [n1, D]
            H1 = (e1 @ w1a).astype(f32)                # [n1, HH]
            h1 = H1.T.reshape(HC, P, n1)               # [hc, p, l]
            b16_host[:, O1:O2].reshape(P, HC, N1)[:, :, :n1] = \
                np.transpose(h1, (1, 0, 2)).astype(f16)
        if n2:
            e2 = table[x2[b][ml]].astype(f64)
            H2 = (e2 @ w1b + tb1.astype(f64)).astype(f32)
            h2 = H2.T.reshape(HC, P, n2)
            b16_host[:, O2:OW].reshape(P, HC, N2P)[:, :, :n2] = \
                np.transpose(h2, (1, 0, 2)).astype(f16)
        vld = np.zeros((NBLK, NPS), dtype=f32)
        if n1 and n2:
            vm = (mask1[b][ll][None, :] != 0) & (mask2[b][ml][:, None] != 0) \
                 & (x1[b][ll][None, :] != x2[b][ml][:, None])   # [n2, n1]
            grid = np.zeros((N2P, N1), dtype=f32)
            grid[:n2, :n1] = vm.astype(f32)
            vld[:, :NP] = grid.reshape(NBLK, K * N1)
        in_maps.append({
            "b16": b16_host,
            "b8": b8_host,
            "bac": bac_host,
            "vld": vld.reshape(1, NBLK * NPS),
        })
    ab2_f = float(np.asarray(ab2).ravel()[0])
    cb_f = float(np.asarray(cb).ravel()[0])
    return (N1, N2P, K, NBLK, ab2_f, cb_f, t_c), in_maps


def kernel(x1, x2, mask1, mask2, embed_table, tw1, tb1, tw2, tb2,
           aw1, ab1, aw2, ab2, cw, cb):
    from concourse import bass_utils

    key_args, in_maps = _prep(
        x1, x2, mask1, mask2, embed_table, tw1, tb1, tw2, tb2,
        aw1, ab1, aw2, ab2, cw, cb)

    if key_args not in _prog_cache:
        _prog_cache[key_args] = _build_program(*key_args)
    nc = _prog_cache[key_args]

    res = bass_utils.run_bass_kernel_spmd(nc, in_maps, core_ids=list(range(8)))
    y = np.stack([res.results[i]["y"].reshape(()) for i in range(B)])
    return y.reshape(B, 1).astype(np.float32)
